# revision 4
# baseline (speedup 1.0000x reference)
"""Trainium2 Bass kernel v4 for nn_LocalInteractionLayer (sparse_attention).

Math per (s, h): softmax over 16 chunk-dots of key row s+h (padded):
  scores[s,h,w] = <q[s, h*64:], k[s+h, w*64:]> / 8
  out[s, h*64+df] = sum_w softmax(scores)[w] * v[s+h, w*64+df]

Sharding: 8 cores = 4 batches x 2 sequence halves (1024 query rows each).

Design (v4):
  - Attention is DVE-bound (~38us per 128-key tile); every big DVE op runs
    in the 2x bf16 mode. Other engines are kept LIGHT: extra ACT/gpsimd
    traffic measurably slows co-running DVE ops (shared SBUF bandwidth).
  - q round-trips through a DRAM scratch; ONE gather DMA per key tile with
    an h-dependent stride (64-1024) builds the shifted per-head q tile.
  - ONE store DMA per tile with the same stride trick (bf16 output).
  - Softmax: no max-subtraction (scores bounded ~|2|).
  - Key chunk 8 has only 120 valid (row, head) slots (pi < h): packed into
    120 partitions via per-h DMAs instead of a full-cost 9th tile. Its
    projections + gathers are emitted mid-stream so the in-order PE/ACT/SP
    queues have them ready long before the tail computes.
"""

import os
import sys

import numpy as np

for _p in ("/opt/trn_rl_repo", "/opt/trn_rl_repo/concourse"):
    if _p not in sys.path and os.path.isdir(_p):
        sys.path.insert(0, _p)

import ml_dtypes

import concourse.bass as bass
import concourse.tile as tile
from concourse import mybir
from concourse.bass_utils import run_bass_kernel_spmd

BF16 = mybir.dt.bfloat16
F32 = mybir.dt.float32

B, S, D = 4, 2048, 1024
WIN, H, DF = 16, 16, 64
HD = H * DF
SH = S // 2            # per-core query rows
HALO = WIN - 1         # 15
NPAD = 1152            # 9 * 128 padded key rows
NT = NPAD // 128       # 9 key chunks (8 full attention tiles + packed tail)
QROWS = NPAD           # qflat scratch rows
NTAIL = 120            # valid (pi, h) slots in the tail tile: pi < h

_CACHE = {}


def build_nc():
    from concourse import bacc
    nc = bacc.Bacc("TRN2", target_bir_lowering=False, debug=False, num_devices=8)

    xT = nc.dram_tensor("xT", [D, NPAD], BF16, kind="ExternalInput")
    wT = nc.dram_tensor("wT", [3, D, HD], BF16, kind="ExternalInput")
    biases = nc.dram_tensor("biases", [3, HD], BF16, kind="ExternalInput")
    # "key-major" raw output: raw[t, h*64+df] = out for query s = t-h.
    # The host de-interleaves (out[s, hcols] = raw[s+h, hcols]); junk slots
    # (s < 0) land in rows/cols the host never reads.
    out = nc.dram_tensor("out", [NPAD, HD], BF16, kind="ExternalOutput")

    with tile.TileContext(nc) as tc:
        _build_tile(tc, xT, wT, biases, out)
    nc.finalize()
    return nc


def _build_tile(tc, xT, wT, biases, out):
    nc = tc.nc
    from contextlib import ExitStack

    with ExitStack() as ctx:
        consts = ctx.enter_context(tc.tile_pool(name="consts", bufs=1))
        dram = ctx.enter_context(tc.tile_pool(name="dram", bufs=1, space="DRAM"))
        qstage = ctx.enter_context(tc.tile_pool(name="qstage", bufs=2))
        ppool = ctx.enter_context(tc.tile_pool(name="ppool", bufs=3, space="PSUM"))
        prod = ctx.enter_context(tc.tile_pool(name="prod", bufs=2))
        qshp = ctx.enter_context(tc.tile_pool(name="qshp", bufs=2))
        smp = ctx.enter_context(tc.tile_pool(name="smp", bufs=2))
        op = ctx.enter_context(tc.tile_pool(name="op", bufs=2))
        tailp = ctx.enter_context(tc.tile_pool(name="tailp", bufs=1))

        # ---- static SBUF ----
        w_sb = consts.tile([128, 3, 8, HD], BF16)     # 48KB/part
        xt = consts.tile([128, 8, NPAD], BF16)        # 18KB/part
        k_sb = consts.tile([128, NT, HD], BF16)       # 18KB/part
        v_sb = consts.tile([128, NT, HD], BF16)       # 18KB/part
        bias_sb = consts.tile([1, 3, HD], BF16)
        ones_sb = consts.tile([1, 128], BF16)

        qflat = dram.tile([QROWS, HD], BF16)

        # ---- setup DMAs (split across SP/ACT queues) ----
        src = bass.AP(
            tensor=xT, offset=0,
            ap=[[NPAD, 128], [128 * NPAD, 8], [1, NPAD]],
        )
        nc.sync.dma_start(out=xt[:], in_=src)
        nc.scalar.dma_start(out=bias_sb[0:1, :, :], in_=biases[:, :])
        for p in range(3):
            eng = (nc.scalar, nc.sync, nc.scalar)[p]
            src = bass.AP(
                tensor=wT, offset=p * D * HD,
                ap=[[HD, 128], [128 * HD, 8], [1, HD]],
            )
            eng.dma_start(out=w_sb[:, p, :, :], in_=src)
        nc.vector.memset(ones_sb[:], 1.0)

        def proj_group(p, c, dst_sb):
            """One projection chunk: rows c*128..c*128+127 of q/k/v."""
            for n0 in (0, 512):
                ps = ppool.tile([128, 512], F32, tag="ps")
                nc.tensor.matmul(
                    ps[:], lhsT=ones_sb[:, :], rhs=bias_sb[:, p, n0:n0 + 512],
                    start=True, stop=False,
                )
                for dc in range(8):
                    nc.tensor.matmul(
                        ps[:],
                        lhsT=xt[:, dc, c * 128:(c + 1) * 128],
                        rhs=w_sb[:, p, dc, n0:n0 + 512],
                        start=False, stop=(dc == 7),
                    )
                nc.scalar.copy(dst_sb[:, n0:n0 + 512], ps[:])

        def q_chunk(c):
            qs = qstage.tile([128, HD], BF16, tag="qs")
            proj_group(0, c, qs)
            qdst = bass.AP(tensor=qflat.tensor, offset=qflat.offset + c * 128 * HD,
                           ap=[[HD, 128], [1, HD]])
            nc.scalar.dma_start(out=qdst, in_=qs[:])

        # ---- attention front: gather + A-mul + A-tree + exp ----
        def att_front(j, st):
            qsh = qshp.tile([128, H, DF], BF16, tag="qsh")
            gsrc = bass.AP(
                tensor=qflat.tensor,
                offset=qflat.offset + (j * 128 + HALO) * HD,
                ap=[[HD, 128], [DF - HD, H], [1, DF]],
            )
            nc.sync.dma_start(out=qsh[:], in_=gsrc)

            prodA = prod.tile([128, H, WIN, DF], BF16, tag="prod")
            kb = k_sb[:, j, :]
            k_view = bass.AP(
                tensor=kb.tensor, offset=kb.offset,
                ap=[list(kb.ap[0]), [0, H], [DF, WIN], [1, DF]],
            )
            q_view = bass.AP(
                tensor=qsh.tensor, offset=qsh.offset,
                ap=[list(qsh.ap[0]), [DF, H], [0, WIN], [1, DF]],
            )
            nc.vector.tensor_mul(prodA[:], k_view, q_view)

            # A-tree (DVE, 2x): df 64 -> 2 in-place, final -> f32 scores
            sz = DF // 2
            while sz >= 2:
                nc.vector.tensor_add(
                    prodA[:, :, :, 0:sz],
                    prodA[:, :, :, 0:sz],
                    prodA[:, :, :, sz:2 * sz],
                )
                sz //= 2
            scr = smp.tile([128, H, WIN], F32, tag="scr")
            nc.vector.tensor_add(scr[:], prodA[:, :, :, 0], prodA[:, :, :, 1])

            e = smp.tile([128, H, WIN], BF16, tag="e")
            nc.scalar.activation(e[:], scr[:], mybir.ActivationFunctionType.Exp)
            st["e"] = e

        # ---- attention back 1: softmax-normalize + x4 attn replication ----
        def att_back1(j, st):
            e = st["e"]
            ssum = smp.tile([128, H], F32, tag="ssum")
            nc.vector.tensor_reduce(ssum[:], e[:], op=mybir.AluOpType.add,
                                    axis=mybir.AxisListType.X)
            recip = smp.tile([128, H], F32, tag="recip")
            nc.vector.reciprocal(recip[:], ssum[:])
            # normalize directly into the x4-replicated layout (1x, but lets
            # every C-tree level run 2x with df innermost)
            attn_x = smp.tile([128, H, WIN, 4], BF16, tag="attn_x", bufs=3)
            e_rep = bass.AP(
                tensor=e.tensor, offset=e.offset,
                ap=[list(e.ap[0]), [WIN, H], [1, WIN], [0, 4]],
            )
            recip_b = bass.AP(
                tensor=recip.tensor, offset=recip.offset,
                ap=[list(recip.ap[0]), [1, H], [0, WIN], [0, 4]],
            )
            nc.vector.tensor_mul(attn_x[:], e_rep, recip_b)
            st["attn_x"] = attn_x

        # ---- attention back 2: C stage (all levels 2x) + store ----
        def att_back2(j, st):
            attn_x = st["attn_x"]
            # C product in [p, h, w, df] (df innermost, PLAIN v)
            prodC = prod.tile([128, H, WIN, DF], BF16, tag="prod")
            vb = v_sb[:, j, :]
            pc_dst = bass.AP(
                tensor=prodC.tensor, offset=prodC.offset,
                ap=[list(prodC.ap[0]), [WIN * DF, H], [DF, WIN], [4, 16], [1, 4]],
            )
            v_view = bass.AP(
                tensor=vb.tensor, offset=vb.offset,
                ap=[list(vb.ap[0]), [0, H], [DF, WIN], [4, 16], [1, 4]],
            )
            ax_view = bass.AP(
                tensor=attn_x.tensor, offset=attn_x.offset,
                ap=[list(attn_x.ap[0]), [WIN * 4, H], [4, WIN], [0, 16], [1, 4]],
            )
            nc.vector.tensor_mul(pc_dst, v_view, ax_view)

            # C-tree (DVE, contiguous sz*64 runs, 2x incl final): w 16 -> 1
            o = op.tile([128, H, DF], BF16, tag="o")
            sz = WIN // 2
            while sz >= 2:
                nc.vector.tensor_add(
                    prodC[:, :, 0:sz, :],
                    prodC[:, :, 0:sz, :],
                    prodC[:, :, sz:2 * sz, :],
                )
                sz //= 2
            nc.vector.tensor_add(o[:], prodC[:, :, 0, :], prodC[:, :, 1, :])

            nc.scalar.dma_start(out=out[j * 128:(j + 1) * 128, :], in_=o[:])

        # ---- packed tail: the 120 valid (pi, h) slots of key chunk 8 ----
        # slot m for (h, pi), pi < h, ordered by h: query s = 1024+pi-h,
        # key/value row = chunk-8 partition pi, q chunk h.
        ksl = tailp.tile([NTAIL, HD], BF16)
        vsl = tailp.tile([NTAIL, HD], BF16)
        qsl = tailp.tile([NTAIL, DF], BF16)

        def tail_gathers():
            for h in range(1, WIN):
                m0 = h * (h - 1) // 2
                e0 = nc.sync if h % 2 else nc.scalar
                e1 = nc.scalar if h % 2 else nc.sync
                e0.dma_start(out=ksl[m0:m0 + h, :], in_=k_sb[0:h, NT - 1, :])
                e1.dma_start(out=vsl[m0:m0 + h, :], in_=v_sb[0:h, NT - 1, :])
                qsrc = bass.AP(
                    tensor=qflat.tensor,
                    offset=qflat.offset + (SH + HALO - h) * HD + h * DF,
                    ap=[[HD, h], [1, DF]],
                )
                e0.dma_start(out=qsl[m0:m0 + h, :], in_=qsrc)

        def tail_compute():
            prodT = tailp.tile([NTAIL, WIN, DF], BF16)
            k_view = bass.AP(
                tensor=ksl.tensor, offset=ksl.offset,
                ap=[list(ksl.ap[0]), [DF, WIN], [1, DF]],
            )
            q_view = bass.AP(
                tensor=qsl.tensor, offset=qsl.offset,
                ap=[list(qsl.ap[0]), [0, WIN], [1, DF]],
            )
            nc.vector.tensor_mul(prodT[:], k_view, q_view)
            sz = DF // 2
            while sz >= 2:
                nc.vector.tensor_add(
                    prodT[:, :, 0:sz], prodT[:, :, 0:sz], prodT[:, :, sz:2 * sz]
                )
                sz //= 2
            scr_t = tailp.tile([NTAIL, WIN], F32)
            nc.vector.tensor_add(scr_t[:], prodT[:, :, 0], prodT[:, :, 1])
            e_t = tailp.tile([NTAIL, WIN], BF16)
            nc.scalar.activation(e_t[:], scr_t[:], mybir.ActivationFunctionType.Exp)
            ssum_t = tailp.tile([NTAIL, 1], F32)
            nc.vector.tensor_reduce(ssum_t[:], e_t[:], op=mybir.AluOpType.add,
                                    axis=mybir.AxisListType.X)
            recip_t = tailp.tile([NTAIL, 1], F32)
            nc.vector.reciprocal(recip_t[:], ssum_t[:])
            attn_t = tailp.tile([NTAIL, WIN], BF16)
            rt_view = bass.AP(
                tensor=recip_t.tensor, offset=recip_t.offset,
                ap=[list(recip_t.ap[0]), [0, WIN]],
            )
            nc.vector.tensor_mul(attn_t[:], e_t[:], rt_view)

            # C: [m, w, df] with plain v; attn broadcast over df (small, 1x ok)
            prodCT = tailp.tile([NTAIL, WIN, DF], BF16)
            vt_view = bass.AP(
                tensor=vsl.tensor, offset=vsl.offset,
                ap=[list(vsl.ap[0]), [DF, WIN], [1, DF]],
            )
            at_view = bass.AP(
                tensor=attn_t.tensor, offset=attn_t.offset,
                ap=[list(attn_t.ap[0]), [1, WIN], [0, DF]],
            )
            nc.vector.tensor_mul(prodCT[:], vt_view, at_view)
            sz = WIN // 2
            while sz >= 2:
                nc.vector.tensor_add(
                    prodCT[:, 0:sz, :], prodCT[:, 0:sz, :], prodCT[:, sz:2 * sz, :]
                )
                sz //= 2
            o_t = tailp.tile([NTAIL, DF], BF16)
            nc.vector.tensor_add(o_t[:], prodCT[:, 0, :], prodCT[:, 1, :])

            for h in range(1, WIN):
                m0 = h * (h - 1) // 2
                odst = bass.AP(
                    tensor=out,
                    offset=SH * HD + h * DF,
                    ap=[[HD, h], [1, DF]],
                )
                eng = nc.sync if h % 2 else nc.scalar
                eng.dma_start(out=odst, in_=o_t[m0:m0 + h, :])

        # ---- software-pipelined emission ----
        # Chunk-8 projections + tail gathers go in mid-stream (after tile 4's
        # front) so the in-order PE/ACT/SP queues complete them early.
        states = [dict() for _ in range(NT - 1)]
        q_chunk(0)
        q_chunk(1)
        proj_group(1, 0, k_sb[:, 0, :])
        proj_group(2, 0, v_sb[:, 0, :])
        att_front(0, states[0])
        for c in range(1, 5):
            q_chunk(c + 1)
            proj_group(1, c, k_sb[:, c, :])
            proj_group(2, c, v_sb[:, c, :])
            att_front(c, states[c])
            att_back1(c - 1, states[c - 1])
            if c >= 2:
                att_back2(c - 2, states[c - 2])
        # mid-stream: remaining q chunks, the tail's projections and gathers
        q_chunk(6)
        q_chunk(7)
        q_chunk(8)
        proj_group(1, 8, k_sb[:, 8, :])
        proj_group(2, 8, v_sb[:, 8, :])
        tail_gathers()
        for c in range(5, 8):
            proj_group(1, c, k_sb[:, c, :])
            proj_group(2, c, v_sb[:, c, :])
            att_front(c, states[c])
            att_back1(c - 1, states[c - 1])
            att_back2(c - 2, states[c - 2])
            if c == 5:
                # tail compute slots in here: its inputs landed with the
                # chunk-8 projections/gathers above, and its 30 small store
                # triggers overlap tiles 6..7 instead of serializing at the
                # very end.
                tail_compute()
        att_back1(7, states[7])
        att_back2(6, states[6])
        att_back2(7, states[7])


def _host_prep(input_seq, Wq, bq, Wk, bk, Wv, bv):
    """Build the 8 per-core input maps."""
    input_seq = np.asarray(input_seq, dtype=np.float32)
    Wq = np.asarray(Wq, dtype=np.float32)
    Wk = np.asarray(Wk, dtype=np.float32)
    Wv = np.asarray(Wv, dtype=np.float32)
    bq = np.asarray(bq, dtype=np.float32)
    bk = np.asarray(bk, dtype=np.float32)
    bv = np.asarray(bv, dtype=np.float32)

    scale = 1.0 / np.sqrt(DF)
    wT = np.stack([
        (Wq.T * scale),
        Wk.T,
        Wv.T,
    ]).astype(ml_dtypes.bfloat16)                    # [3, D, HD]
    biases = np.stack([
        bq * scale,
        bk,
        bv,
    ]).astype(ml_dtypes.bfloat16)                    # [3, HD]

    in_maps = []
    for c in range(8):
        b, half = c // 2, c % 2
        s0 = half * SH
        xh = np.zeros((NPAD, D), dtype=np.float32)
        lo = s0 - HALO
        src_lo = max(lo, 0)
        xh[src_lo - lo: src_lo - lo + (s0 + SH - src_lo)] = input_seq[b, src_lo: s0 + SH]
        xTa = np.ascontiguousarray(xh.T).astype(ml_dtypes.bfloat16)
        in_maps.append({"xT": xTa, "wT": wT, "biases": biases})
    return in_maps


def _get_nc():
    if "nc" not in _CACHE:
        _CACHE["nc"] = build_nc()
    return _CACHE["nc"]


def _ensure_ntff_hook():
    """Register the axon NTFF profile hook if the image's antenv lacks it."""
    import types
    try:
        from antenv.axon_hooks import get_axon_ntff_profile_hook  # noqa: F401
        return
    except ImportError:
        pass
    try:
        import antenv
        mod = types.ModuleType("antenv.axon_hooks")
        _state = {"hook": None}
        mod.set_axon_ntff_profile_hook = lambda h: _state.__setitem__("hook", h)
        mod.get_axon_ntff_profile_hook = lambda: _state["hook"]
        sys.modules["antenv.axon_hooks"] = mod
        antenv.axon_hooks = mod
        boot_dir = "/root/.axon_site/trn_agent_boot"
        if boot_dir not in sys.path and os.path.isdir(boot_dir):
            sys.path.insert(0, boot_dir)
        import trn_boot
        hook = trn_boot._ntff_profile_via_ctypes("/opt/axon/libaxon_pjrt.so")
        if hook is not None:
            mod.set_axon_ntff_profile_hook(hook)
    except Exception as e:  # profiling is best-effort
        print(f"ntff hook setup failed: {e}")


def kernel(input_seq, Wq, bq, Wk, bk, Wv, bv, trace=False, **trace_kwargs):
    if trace:
        _ensure_ntff_hook()
    nc = _get_nc()
    in_maps = _host_prep(input_seq, Wq, bq, Wk, bk, Wv, bv)
    res = run_bass_kernel_spmd(nc, in_maps, list(range(8)), trace=trace, **trace_kwargs)
    out = np.empty((B, S, HD), dtype=np.float32)
    for c in range(8):
        b, half = c // 2, c % 2
        raw = res.results[c]["out"]  # [NPAD, HD] key-major
        dst = out[b, half * SH:(half + 1) * SH]
        for h in range(H):
            cols = slice(h * DF, (h + 1) * DF)
            dst[:, cols] = raw[h:h + SH, cols].astype(np.float32)
    if trace:
        return out, res
    return out
